# revision 56
# baseline (speedup 1.0000x reference)
"""MoE top-2 SwiGLU kernel for TRN2, expert-parallel across 8 NeuronCores.

Strategy:
  - Host: fp32 gating (softmax + top-2, exact replication of the reference),
    dispatch = gather each expert's tokens into a padded [d, C] activation
    block (expert parallelism: core e holds expert e's weights only).
  - Device (per core): split-precision fp8 SwiGLU MLP using DoubleRow
    matmuls (2 contraction planes per instruction at 0.5 cycles/row).
    Every operand is decomposed into e4m3 hi+lo planes (a = ah + al with
    ~7 effective mantissa bits); each K=128 logical contraction chunk
    issues 3 planes (ah*bh, al*bh, ah*bl), pairing planes of adjacent
    chunks into DoubleRow instructions. All three products share one
    power-of-2 scale so they accumulate in a single PSUM group; the
    descale folds into the PSUM readout. 0.75 cycles/row per logical
    chunk vs 1.0 for bf16, with bf16-or-better accuracy (~1.8e-3).
        h = silu(W1 @ x) * (W3 @ x);  out = W2 @ h
    computed entirely transposed ([feature, token] layout) so both matmul
    stages contract on the partition dim with zero on-device transposes.
  - Host: combine = scatter-add weighted expert outputs (fp32).
"""

import numpy as np
import ml_dtypes

import concourse.bacc as bacc
import concourse.mybir as mybir
import concourse.tile as tile
from concourse.bass_utils import run_bass_kernel_spmd

F8 = mybir.dt.float8e4
F32 = mybir.dt.float32
f8 = ml_dtypes.float8_e4m3
DR = mybir.MatmulPerfMode.DoubleRow

NUM_EXPERTS = 8
TOP_K = 2
D_MODEL = 1024
D_MLP = 3584
KD = D_MODEL // 128  # 8 contraction chunks over d_model
FC = D_MLP // 128    # 28 chunks over d_mlp

# Power-of-2 quantization scales (descale folded into PSUM readout).
SX = 16.0    # x ~ N(0,1)
SW = 1024.0  # W ~ N(0, 0.02^2)
SH = 16.0    # h = silu(h1)*h3, |h|max ~ 8.5
DS1 = 1.0 / (SX * SW)   # psum -> true h1/h3
CH = SH / (SX * SW)     # psum-scaled hf -> SH*h
DS2 = 1.0 / (SH * SW)   # stage-2 psum -> true out

# Plane-type order per chunk pair: (w_plane, x_plane) with 0=hi, 1=lo.
PLANES = ((0, 0), (0, 1), (1, 0))

# Populated after each kernel() call so test.py can report device timing.
LAST_RUN = {}

ACT_FN = mybir.ActivationFunctionType.Silu
COPY = mybir.ActivationFunctionType.Copy
MULT = mybir.AluOpType.mult
SUB = mybir.AluOpType.subtract

PS1_BUFS = 3
PS2_BUFS = 2
W_BUFS = 4
W2_BUFS = 2
PASS_CAP = 1536  # max tokens per core per pass (SBUF residency bound)
WARMUP = 60  # dummy matmuls covering the DMA lead-in + PE p-state ramp


def _t_tiles(C):
    """Equal-ish token tiles <=512 (psum bank limit), multiples of 8."""
    n = -(-C // 512)
    tn = -(-C // n // 8) * 8
    tiles = []
    t0 = 0
    while t0 < C:
        t = min(tn, C - t0)
        tiles.append((t0, t))
        t0 += t
    return tiles


def _build_bass(C, S=0, CB=0):
    """Phase A: expert-parallel MLP over C tokens per core.

    Phase B (if S > 0): the S max-loaded experts' overflow tokens (CB per
    segment, same tokens replicated on every core) are processed
    tensor-parallel over the d_mlp chunk axis: each core covers 4 fc-chunk
    slots (host packs which fc's weights go in each core's slots; unused
    slots are zero). Partial outputs are summed across cores on the host.
    Phase-B instructions interleave into phase A's stream so the extra PE
    time is just the (tiny) overflow matmul work.
    """
    t_tiles = _t_tiles(C)
    nc = bacc.Bacc("TRN2", target_bir_lowering=False, debug=False,
                   num_devices=NUM_EXPERTS)

    xq_d = nc.dram_tensor("xq", [KD, 128, 2, C], F8, kind="ExternalInput")
    w13_d = nc.dram_tensor("w13q", [FC, 128, 2, KD, 2, 128], F8,
                           kind="ExternalInput")
    w2_d = nc.dram_tensor("w2q", [KD, 128, FC, 2, 128], F8, kind="ExternalInput")
    out_d = nc.dram_tensor("out", [KD, 128, C], F32, kind="ExternalOutput")
    if S:
        xb_d = nc.dram_tensor("xb", [S, 128, KD, 2, CB], F8,
                              kind="ExternalInput")
        w13b_d = nc.dram_tensor("w13b", [S, 128, 4, 2, KD, 2, 128], F8,
                                kind="ExternalInput")
        w2b_d = nc.dram_tensor("w2b", [S, 128, KD, 4, 2, 128], F8,
                               kind="ExternalInput")
        outb_d = nc.dram_tensor("outb", [S, 128, KD * CB], F32,
                                kind="ExternalOutput")

    with tile.TileContext(nc) as tc:
        with (
            tc.tile_pool(name="xpool", bufs=1) as xpool,
            tc.tile_pool(name="wpool", bufs=W_BUFS) as wpool,
            tc.tile_pool(name="w2pool", bufs=W2_BUFS) as w2pool,
            tc.tile_pool(name="hpool", bufs=1) as hpool,
            tc.tile_pool(name="spool", bufs=4) as spool,
            tc.tile_pool(name="fpool", bufs=4) as fpool,
            tc.tile_pool(name="opool", bufs=4) as opool,
            tc.tile_pool(name="ps1", bufs=PS1_BUFS, space="PSUM") as ps1,
            tc.tile_pool(name="ps2", bufs=PS2_BUFS, space="PSUM") as ps2,
        ):
            # PE warmup on a zeroed scratch: keeps the tensor engine busy
            # (and its p-state ramping) while the first weight/activation
            # DMAs stream in. Output psum is never read.
            scr = xpool.tile([128, 2, 128], F8, tag="wu")
            nc.vector.memset(scr[:], 0.0)
            wups = ps2.tile([128, 128], F32, tag="po", name="wups")
            for i in range(WARMUP):
                nc.tensor.matmul(wups[:], scr[:], scr[:],
                                 start=(i == 0), stop=(i == WARMUP - 1),
                                 perf_mode=DR)

            # Resident activations: hi/lo fp8 planes of SX*X^T, [kd][{h,l}][C].
            # fc0 weights first (one fused w1+w3 DMA), then per-chunk X DMAs;
            # matmuls chase the stream chunk-pair by chunk-pair.
            w13_first = wpool.tile([128, 2, KD, 2, 128], F8, tag="w13")
            nc.sync.dma_start(w13_first[:], w13_d[0])
            xt = xpool.tile([128, KD, 2, C], F8, tag="xt")
            for kd in range(KD):
                nc.sync.dma_start(xt[:, kd], xq_d[kd])

            xbs, w13bs, w2bs, hbs, obs = [], [], [], [], []
            zscr = None
            if S:
                zscr = xpool.tile([128, 512], F8, tag="zs")
                nc.vector.memset(zscr[:], 0.0)
            for s in range(S):
                xbs.append(xpool.tile([128, KD, 2, CB], F8, tag=f"xb{s}",
                                      name=f"xb{s}"))
                w13bs.append(xpool.tile([128, 4, 2, KD, 2, 128], F8,
                                        tag=f"w13b{s}", name=f"w13b{s}"))
                w2bs.append(xpool.tile([128, KD, 4, 2, 128], F8,
                                       tag=f"w2b{s}", name=f"w2b{s}"))
                hbs.append([hpool.tile([128, 2, 2, CB], F8, tag=f"hb{s}{j}",
                                       name=f"hb{s}{j}") for j in range(2)])
                obs.append(xpool.tile([128, KD * CB], F32, tag=f"ob{s}",
                                      name=f"ob{s}"))

            def mm_group(psum, w, rhs, rhs_sel, t0, tn, nchunk, accum=False):
                """3-plane DoubleRow accumulation over nchunk logical chunks.

                accum=True: land in an already-opened psum bank (start=False
                everywhere) so several groups can share one bank region-wise.
                """
                order = [(j, wi, xi) for j in range(nchunk // 2)
                         for (wi, xi) in PLANES]
                n_inst = len(order)
                for idx, (j, wi, xi) in enumerate(order):
                    k0 = 2 * j
                    nc.tensor.matmul(
                        psum[:], w[:, k0:k0 + 2, wi, :],
                        rhs_sel(rhs, k0, xi, t0, tn),
                        start=(idx == 0 and not accum), stop=(idx == n_inst - 1),
                        perf_mode=DR, skip_group_check=accum)

            def x_sel(r, k0, xi, t0, tn):
                return r[:, k0:k0 + 2, xi, t0:t0 + tn]

            # Stage 1: h planes, one [fc-pair][{fc0,fc1}][{h,l}][C] tile each.
            hts = [hpool.tile([128, 2, 2, C], F8, tag=f"h{j}", name=f"h{j}")
                   for j in range(FC // 2)]
            def swiglu_chain(p1, p3, ht, hr, t0, tn):
                s = spool.tile([128, tn], F32, tag="s", name="s")
                nc.scalar.activation(s[:], p1[:], ACT_FN, scale=DS1)
                hf = fpool.tile([128, tn], F32, tag="hf", name="hf")
                nc.vector.tensor_mul(hf[:], s[:], p3[:])
                hh = ht[:, hr, 0, t0:t0 + tn]
                nc.scalar.activation(hh, hf[:], COPY, scale=CH)
                nc.vector.scalar_tensor_tensor(
                    ht[:, hr, 1, t0:t0 + tn], hf[:], CH, hh, MULT, SUB)

            bseg_psum = {}

            def phaseb_slot(s, slot):
                """Stage-1 of phase B for one fc slot of segment s.

                All 4 slots of a segment share one psum bank per matrix
                (stage-2 ring, idle during stage 1): a zeroing matmul opens
                the bank at slot 0, then start=False sub-groups land in
                disjoint regions — no ring waits, so phase A never stalls.
                """
                if slot == 0:
                    bp1 = ps2.tile([128, 4 * CB], F32, tag="po", name="bp1")
                    bp3 = ps2.tile([128, 4 * CB], F32, tag="po", name="bp3")
                    for bp in (bp1, bp3):
                        nc.tensor.matmul(bp[:], zscr[:, 0:128],
                                         zscr[:, 0:4 * CB],
                                         start=True, stop=True)
                    bseg_psum[s] = (bp1, bp3)
                bp1, bp3 = bseg_psum[s]
                c0 = slot * CB
                p1 = bp1[:, c0:c0 + CB]
                p3 = bp3[:, c0:c0 + CB]
                mm_group(p1, w13bs[s][:, slot, 0], xbs[s], x_sel, 0, CB, KD,
                         accum=True)
                mm_group(p3, w13bs[s][:, slot, 1], xbs[s], x_sel, 0, CB, KD,
                         accum=True)
                swiglu_chain(p1, p3, hbs[s][slot // 2], slot % 2, 0, CB)

            b_blocks = [(s, slot) for s in range(S) for slot in range(4)]
            for fc in range(FC):
                if fc == 0:
                    w13 = w13_first
                else:
                    w13 = wpool.tile([128, 2, KD, 2, 128], F8, tag="w13")
                    nc.sync.dma_start(w13[:], w13_d[fc])
                if fc == 4:
                    for s in range(S):
                        nc.sync.dma_start(xbs[s][:], xb_d[s])
                if fc == 8:
                    for s in range(S):
                        nc.sync.dma_start(w13bs[s][:], w13b_d[s])
                if fc == 12:
                    for s in range(S):
                        nc.sync.dma_start(w2bs[s][:], w2b_d[s])
                w1, w3 = w13[:, 0], w13[:, 1]
                hj, hr = divmod(fc, 2)
                ht = hts[hj]
                for (t0, tn) in t_tiles:
                    p1 = ps1.tile([128, tn], F32, tag="p1")
                    p3 = ps1.tile([128, tn], F32, tag="p3")
                    mm_group(p1, w1, xt, x_sel, t0, tn, KD)
                    mm_group(p3, w3, xt, x_sel, t0, tn, KD)
                    swiglu_chain(p1, p3, ht, hr, t0, tn)
                # interleave phase-B stage-1 slots, one per fc round
                if 0 <= fc - 14 < len(b_blocks):
                    phaseb_slot(*b_blocks[fc - 14])

            def h_sel(r, k0, xi, t0, tn):
                # r is hts; k0 = 2*j -> pair tile j, planes = both fc's.
                return hts[k0 // 2][:, :, xi, t0:t0 + tn]

            # Stage 2: out^T[dc] = sum_fc W2^T[fc,dc] @ h^T[fc]
            for dc in range(KD):
                w2 = w2pool.tile([128, FC, 2, 128], F8, tag="w2",
                                 name=f"w2_{dc}")
                nc.sync.dma_start(w2[:], w2_d[dc])
                if dc == 2 and S:
                    # Phase-B stage 2 runs here, overlapping the first w2
                    # stream: partial out over this core's 4 fc slots. All 8
                    # output chunks of a segment accumulate into one psum
                    # bank: a zeroing matmul opens the bank, then start=False
                    # sub-groups write disjoint regions.
                    def hb_sel(r, k0, xi, t0, tn):
                        return hbs[r][k0 // 2][:, :, xi, t0:t0 + tn]
                    for s in range(S):
                        pob = ps2.tile([128, KD * CB], F32, tag="po",
                                       name=f"pob{s}")
                        nc.tensor.matmul(pob[:], zscr[:, 0:128],
                                         zscr[:, 0:KD * CB],
                                         start=True, stop=True)
                        for bdc in range(KD):
                            mm_group(pob[:, bdc * CB:(bdc + 1) * CB],
                                     w2bs[s][:, bdc], s, hb_sel, 0, CB, 4,
                                     accum=True)
                        nc.vector.tensor_scalar_mul(obs[s][:], pob[:], DS2)
                        nc.sync.dma_start(outb_d[s], obs[s][:])
                tiles = t_tiles
                if dc == KD - 1:
                    # halve the final tile so the tail copy+DMA is short
                    (t0, tn) = t_tiles[-1]
                    h1 = tn // 2 // 8 * 8
                    tiles = t_tiles[:-1] + [(t0, h1), (t0 + h1, tn - h1)]
                for (t0, tn) in tiles:
                    po = ps2.tile([128, tn], F32, tag="po")
                    mm_group(po, w2, None, h_sel, t0, tn, FC)
                    ot = opool.tile([128, tn], F32, tag="o")
                    nc.vector.tensor_scalar_mul(ot[:], po[:], DS2)
                    nc.sync.dma_start(out_d[dc][:, t0:t0 + tn], ot[:])

    nc.compile()
    return nc


def _gate(xt, W_gate):
    """fp32 softmax top-2 gating, matching jax.lax.top_k tie-breaking."""
    logits = xt @ W_gate.T
    m = logits.max(-1, keepdims=True)
    ex = np.exp(logits - m)
    w = ex / ex.sum(-1, keepdims=True)
    top_i = np.argsort(-w, axis=-1, kind="stable")[:, :TOP_K]
    top_w = np.take_along_axis(w, top_i, -1)
    top_w = top_w / top_w.sum(-1, keepdims=True)
    return top_i, top_w.astype(np.float32)


def _split8(a, scale):
    """hi/lo e4m3 planes of scale*a (fp32 in, fp8 pair out)."""
    s = (scale * a).astype(np.float32)
    hi = s.astype(f8)
    lo = (s - hi.astype(np.float32)).astype(f8)
    return hi, lo


def _pack_w13(W):
    """[F, D] -> [FC, 128p(d), KD, 2{h,l}, 128m(f)] fp8."""
    hi, lo = _split8(W, SW)
    out = np.empty((FC, 128, KD, 2, 128), dtype=f8)
    for i, a in enumerate((hi, lo)):
        # a[fc*128+m, kd*128+p] -> out[fc, p, kd, i, m]
        out[:, :, :, i, :] = a.reshape(FC, 128, KD, 128).transpose(0, 3, 2, 1)
    return np.ascontiguousarray(out)


def _pack_w2(W):
    """[D, F] -> [KD, 128p(f), FC, 2{h,l}, 128m(d)] fp8."""
    hi, lo = _split8(W, SW)
    out = np.empty((KD, 128, FC, 2, 128), dtype=f8)
    for i, a in enumerate((hi, lo)):
        # a[dc*128+m, fc*128+p] -> out[dc, p, fc, i, m]
        out[:, :, :, i, :] = a.reshape(KD, 128, FC, 128).transpose(0, 3, 2, 1)
    return np.ascontiguousarray(out)


def _pack_x(XT, C):
    """[D, n] (n<=C) -> [KD, 128, 2{h,l}, C] fp8, zero padded."""
    out = np.zeros((KD, 128, 2, C), dtype=f8)
    hi, lo = _split8(XT, SX)
    n = XT.shape[1]
    out[:, :, 0, :n] = hi.reshape(KD, 128, n)
    out[:, :, 1, :n] = lo.reshape(KD, 128, n)
    return out


def kernel(x, W_gate, W1, W3, W2):
    x = np.asarray(x, dtype=np.float32)
    W_gate = np.asarray(W_gate, dtype=np.float32)
    W1 = np.asarray(W1, dtype=np.float32)
    W3 = np.asarray(W3, dtype=np.float32)
    W2 = np.asarray(W2, dtype=np.float32)

    B, P, D = x.shape
    T = B * P
    xt = x.reshape(T, D)

    top_i, top_w = _gate(xt, W_gate)

    idxs, wts = [], []
    for e in range(NUM_EXPERTS):
        rows, slots = np.nonzero(top_i == e)
        idxs.append(rows)
        wts.append(top_w[rows, slots])

    counts = [len(i) for i in idxs]
    max_count = max(counts)
    n_pass = max(1, -(-max_count // PASS_CAP))

    # Phase-B plan: cap phase-A capacity at the (cut+1)-th largest expert
    # count and offload the overflow of the `cut` largest experts to the
    # TP phase. Costs in PE cycles; 72 = phase-B cycles per overflow token
    # (1/8 of the per-token MLP), ~3500 = per-segment pipeline overhead.
    C_A = max_count
    if n_pass == 1:
        best = 504 * max_count
        for cut in (1, 2):
            ca = sorted(counts, reverse=True)[cut]
            sg = [e for e in range(NUM_EXPERTS) if counts[e] > ca]
            if not sg or len(sg) > 2:
                continue
            cb = -(-max(counts[e] - ca for e in sg) // 8) * 8
            if cb > 512 // KD:  # segment psum must fit one bank
                continue
            cost = 504 * ca + 72 * len(sg) * cb + 3500 * len(sg)
            if cost < best:
                best, C_A = cost, ca
        C = max(128, -(-C_A // 8) * 8)
    else:
        cap = -(-max_count // n_pass)
        C = max(128, -(-cap // 16) * 16)
    segs = [e for e in range(NUM_EXPERTS) if counts[e] > C] if n_pass == 1 else []
    S = len(segs)
    CB = (-(-max(counts[e] - C for e in segs) // 8) * 8) if S else 0
    if S > 2 or CB > 512 // KD:
        segs, S, CB = [], 0, 0
        C = max(128, -(-max_count // 16) * 16)

    wt_maps = [{"w13q": np.ascontiguousarray(
                    np.stack([_pack_w13(W1[e]), _pack_w13(W3[e])], axis=2)),
                "w2q": _pack_w2(W2[e])} for e in range(NUM_EXPERTS)]

    bt_maps = [dict() for _ in range(NUM_EXPERTS)]
    if S:
        xb = np.stack([
            _pack_x(xt[idxs[e][C:C + CB]].T, CB).transpose(1, 0, 2, 3)
            for e in segs])  # [S, 128, KD, 2, CB]
        for k in range(NUM_EXPERTS):
            w13b = np.zeros((S, 128, 4, 2, KD, 2, 128), dtype=f8)
            w2b = np.zeros((S, 128, KD, 4, 2, 128), dtype=f8)
            for s, e in enumerate(segs):
                for j in range(4):
                    fc = k + 8 * j
                    if fc < FC:
                        w13b[s, :, j] = wt_maps[e]["w13q"][fc]
                        w2b[s, :, :, j] = (
                            wt_maps[e]["w2q"][:, :, fc].transpose(1, 0, 2, 3))
            bt_maps[k] = {"xb": xb, "w13b": w13b, "w2b": w2b}

    nc = _build_bass(C, S, CB)
    out = np.zeros((T, D), dtype=np.float32)
    for p in range(n_pass):
        in_maps = []
        for e in range(NUM_EXPERTS):
            sel = idxs[e][p * C:(p + 1) * C]
            in_maps.append({"xq": _pack_x(xt[sel].T, C),
                            **wt_maps[e], **bt_maps[e]})
        res = run_bass_kernel_spmd(nc, in_maps, list(range(NUM_EXPERTS)))
        LAST_RUN["results"] = res
        LAST_RUN["C"] = C
        LAST_RUN["nc"] = nc
        LAST_RUN["ncs"] = [nc]
        LAST_RUN["in_maps"] = in_maps
        for e in range(NUM_EXPERTS):
            sel = idxs[e][p * C:(p + 1) * C]
            if len(sel):
                O = np.asarray(res.results[e]["out"]).reshape(D, C)
                w_sel = wts[e][p * C:(p + 1) * C]
                out[sel] += w_sel[:, None] * O[:, :len(sel)].T
        for s, e in enumerate(segs):
            sel = idxs[e][C:C + CB]
            OB = sum(np.asarray(res.results[k]["outb"])[s]
                     for k in range(NUM_EXPERTS))  # [128, KD*CB]
            OB = OB.reshape(128, KD, CB).transpose(1, 0, 2).reshape(D, CB)
            out[sel] += wts[e][C:C + len(sel)][:, None] * OB[:, :len(sel)].T
    return out.reshape(B, P, D)
